# revision 5
# baseline (speedup 1.0000x reference)
"""Multi-head attention (B=4, S=2048, D=512, H=8) on 8 Trainium2 cores.

Sharding: core c = (batch b = c//2, query-half = c%2). Each core computes
1024 query rows of one batch over all 2048 keys and all 8 heads, producing
a disjoint slice of the output -> no inter-core reduction needed.

Per-core layout is fully "transposed land" (contraction dim on partitions):
  xT [512,1024], yT [512,2048] prepared (transposed, bf16) on host.
  QT = Wq^T @ xT   (Wq pre-scaled by depth^-0.5 on host)
  KT = Wk^T @ yT
  V  = y @ Wv in natural [keys, dim] layout, stored strided into
       V_aug [128, 8*65] with a ones column per head (row 64 of the
       attention matmul output then accumulates softmax denominators).

Schedule (v2 — ScalarE exp is the bottleneck engine at ~142us busy, so
everything is organized to start it early and never starve it):
  - DMA priority: wk+yT first, then wq+xT, then wv, wo.
  - Only KT/QT for head pair 0 are computed up front; first exp issues
    ~15us in. V tiles and later pairs' KT/QT projections are emitted
    inside the attention loops where the PE has slack (ScalarE-bound
    steady state leaves ~40% PE idle per iteration).
  - per head pair (2p, 2p+1): head A on partitions 0:64, head B on
    64:128 of shared KT/QT tiles; their logits matmuls target disjoint
    PE row groups and run concurrently.
       logits[kt,qb] = (KT tile)^T @ QT  (bf16 operands, fp32 PSUM)
       exp over [128, 1024] (ScalarE, PSUM -> SBUF bf16)
       attnT += V_aug^T @ PT, fp32 PSUM, accumulated over 16 key tiles.
  - pair-end normalization: evacuate both heads' [65,1024] PSUM to SBUF
    (releases the psum banks for the next pair), then per head:
    reciprocal_approx_fast on the denominator row (single DVE op, ~51
    ULP — vs ~7us for the exact iterative reciprocal), gpsimd
    partition_broadcast, DVE multiply -> attnT bf16. All off the
    critical path except for the last pair.
  - out = attnT^T @ Wo per 128-query tile -> DMA (fp32).
Softmax skips max-subtraction (logits ~ N(0,1); exp cannot overflow fp32).
Matmul operands are bf16 (1 cycle/row on the PE vs 2 for fp32); all PSUM
accumulation fp32. End-to-end RMS relative error vs fp32 ~4e-3.
"""

import numpy as np
import ml_dtypes

import concourse.bass as bass
import concourse.tile as tile
from concourse import bacc, mybir
from concourse.bass_utils import run_bass_kernel_spmd

F32 = mybir.dt.float32
BF16 = mybir.dt.bfloat16
EXP = mybir.ActivationFunctionType.Exp

B, S, D = 4, 2048, 512
H = 8
DEPTH = D // H  # 64
SQ = S // 2  # queries per core (1024)
SK = S  # keys per core (2048)
N_CORES = 8

P = 128
KT4 = D // P  # 4 contraction tiles for projections
NKT = SK // P  # 16 key tiles
NQT = SQ // P  # 8 query tiles
VAUG_W = H * (DEPTH + 1)  # 520


def build_nc():
    nc = bacc.Bacc("TRN2", target_bir_lowering=False, debug=False)

    xT = nc.dram_tensor("xT", [D, SQ], BF16, kind="ExternalInput").ap()
    yT = nc.dram_tensor("yT", [D, SK], BF16, kind="ExternalInput").ap()
    wq = nc.dram_tensor("wq", [D, D], BF16, kind="ExternalInput").ap()
    wk = nc.dram_tensor("wk", [D, D], BF16, kind="ExternalInput").ap()
    wv = nc.dram_tensor("wv", [D, D], BF16, kind="ExternalInput").ap()
    wo = nc.dram_tensor("wo", [D, D], BF16, kind="ExternalInput").ap()
    out = nc.dram_tensor("out", [SQ, D], F32, kind="ExternalOutput").ap()

    with tile.TileContext(nc) as tc:
        with (
            tc.tile_pool(name="acts", bufs=1) as apool,
            tc.tile_pool(name="ps", bufs=1, space="PSUM") as pspool,
            tc.tile_pool(name="pt", bufs=6) as ptpool,
            tc.tile_pool(name="small", bufs=2) as spool,
            tc.tile_pool(name="outsb", bufs=2) as opool,
        ):
            # ---- load inputs, in the order the compute needs them ----
            def load4(name, src, width):
                tiles = []
                for k in range(KT4):
                    t = apool.tile([P, width], BF16, name=f"{name}{k}", tag=f"{name}{k}")
                    nc.sync.dma_start(t[:], src[k * P : (k + 1) * P, :])
                    tiles.append(t)
                return tiles

            wk_sb = load4("wk", wk, D)
            yT_sb = load4("yt", yT, SK)
            wq_sb = load4("wq", wq, D)
            xT_sb = load4("xt", xT, SQ)
            wv_sb = load4("wv", wv, D)
            wo_sb = load4("wo", wo, D)

            ones_sb = apool.tile([P, H], F32, name="ones_sb", tag="ones", bufs=1)
            nc.vector.memset(ones_sb[:], 1.0)
            ones_v = ones_sb.rearrange("p (h c) -> p h c", h=H, c=1)

            # ---- projection emitters (each borrows one 'lg' psum slot) ----
            V_sb = [None] * NKT

            def emit_v(kt):
                t = apool.tile([P, VAUG_W], BF16, name=f"vaug{kt}", tag=f"vaug{kt}")
                ps = pspool.tile([P, SQ], F32, name=f"vps{kt}", tag="lg", bufs=2)
                for k in range(KT4):
                    nc.tensor.matmul(
                        ps[:, :512],
                        yT_sb[k][:, kt * P : (kt + 1) * P],
                        wv_sb[k][:],
                        start=(k == 0),
                        stop=(k == KT4 - 1),
                    )
                tv = t.rearrange("p (h c) -> p h c", h=H, c=DEPTH + 1)
                nc.vector.tensor_copy(
                    tv[:, :, 0:DEPTH],
                    ps[:, :512].rearrange("p (h c) -> p h c", h=H, c=DEPTH),
                )
                nc.vector.tensor_copy(tv[:, :, DEPTH : DEPTH + 1], ones_v)
                V_sb[kt] = t

            QT_sb = [None] * KT4
            KT_sb = [None] * KT4

            def emit_kt_half(p, kb):
                if KT_sb[p] is None:
                    KT_sb[p] = apool.tile(
                        [P, SK], BF16, name=f"ktsb{p}", tag=f"ktsb{p}"
                    )
                t = KT_sb[p]
                ps = pspool.tile([P, SQ], F32, name=f"ktps{p}_{kb}", tag="lg", bufs=2)
                for qb in range(2):
                    for k in range(KT4):
                        nc.tensor.matmul(
                            ps[:, qb * 512 : (qb + 1) * 512],
                            wk_sb[k][:, p * P : (p + 1) * P],
                            yT_sb[k][
                                :,
                                kb * SQ + qb * 512 : kb * SQ + (qb + 1) * 512,
                            ],
                            start=(k == 0),
                            stop=(k == KT4 - 1),
                        )
                nc.vector.tensor_copy(t[:, kb * SQ : (kb + 1) * SQ], ps[:])

            def emit_qt(p):
                ps = pspool.tile([P, SQ], F32, name=f"qtps{p}", tag="lg", bufs=2)
                for qb in range(SQ // 512):
                    for k in range(KT4):
                        nc.tensor.matmul(
                            ps[:, qb * 512 : (qb + 1) * 512],
                            wq_sb[k][:, p * P : (p + 1) * P],
                            xT_sb[k][:, qb * 512 : (qb + 1) * 512],
                            start=(k == 0),
                            stop=(k == KT4 - 1),
                        )
                t = apool.tile([P, SQ], BF16, name=f"qtsb{p}", tag=f"qtsb{p}")
                nc.vector.tensor_copy(t[:], ps[:])
                QT_sb[p] = t

            # ---- prologue: only pair 0's KT/QT, plus the first V tiles ----
            emit_kt_half(0, 0)
            emit_kt_half(0, 1)
            emit_qt(0)
            emit_v(0)
            emit_v(1)

            attnT_sb = []
            for p in range(KT4):
                t = apool.tile([P, SQ], BF16, name=f"attnt{p}", tag=f"attnt{p}")
                attnT_sb.append(t)

            # ---- attention, head-pair by head-pair ----
            for pr in range(KT4):
                attn_pair = []
                for half in range(2):
                    h = 2 * pr + half
                    t = pspool.tile(
                        [DEPTH + 1, SQ], F32, name=f"attnps{h}", tag="at", bufs=2
                    )
                    attn_pair.append(t)
                for kt in range(NKT):
                    pts = []
                    for qb in range(2):
                        lg = pspool.tile(
                            [P, SQ], F32, name=f"lg{pr}_{kt}_{qb}", tag="lg", bufs=2
                        )
                        for half in range(2):
                            nc.tensor.matmul(
                                lg[:, half * 512 : (half + 1) * 512],
                                KT_sb[pr][
                                    half * DEPTH : (half + 1) * DEPTH,
                                    kt * P : (kt + 1) * P,
                                ],
                                QT_sb[pr][
                                    half * DEPTH : (half + 1) * DEPTH,
                                    qb * 512 : (qb + 1) * 512,
                                ],
                                start=True,
                                stop=True,
                            )
                        pt = ptpool.tile(
                            [P, SQ], BF16, name=f"pt{pr}_{kt}_{qb}", tag="pt"
                        )
                        nc.scalar.activation(pt[:], lg[:], EXP)
                        pts.append(pt)

                        # Fill the PE's wait-for-exp window with projection
                        # work (the engine queue is strict FIFO, so these
                        # must sit between the logits and the PV matmuls).
                        if qb == 0:
                            if pr == 0 and 2 <= kt + 2 < NKT:
                                emit_v(kt + 2)
                            if pr + 1 < KT4:
                                if kt == 5:
                                    emit_kt_half(pr + 1, 0)
                                elif kt == 9:
                                    emit_kt_half(pr + 1, 1)
                                elif kt == 13:
                                    emit_qt(pr + 1)

                        for half in range(2):
                            h = 2 * pr + half
                            nc.tensor.matmul(
                                attn_pair[half][:, qb * 512 : (qb + 1) * 512],
                                V_sb[kt][
                                    :, h * (DEPTH + 1) : (h + 1) * (DEPTH + 1)
                                ],
                                pt[:, half * 512 : (half + 1) * 512],
                                start=(kt == 0),
                                stop=(kt == NKT - 1),
                            )
                # ---- pair-end normalization ----
                # Evacuate both heads' PSUM first (releases the 'at' banks so
                # the next pair's PV matmuls can start). Pairs 0-2 then run a
                # single batched DVE reciprocal (both heads' denominator rows
                # gathered to partitions 0/32 of one tile; the iterative
                # reciprocal is free-dim-bound, so one [33,1024] op costs the
                # same as [1,1024]) off the critical path. The last pair uses
                # the ScalarE exp(-ln(x)) chain instead (same table set as
                # the softmax exp, ~9e-6 rel err): ScalarE is idle once the
                # final exp retires, and its chain is much shorter than the
                # ~8.5us DVE reciprocal, shrinking the kernel tail.
                last = pr == KT4 - 1
                auns = []
                if last:
                    ln_den = spool.tile([1, SQ], F32, name="ln_den", tag="lnden", bufs=2)
                    nc.scalar.activation(
                        ln_den[:], attn_pair[0][DEPTH : DEPTH + 1, :],
                        mybir.ActivationFunctionType.Ln,
                    )
                    ln_den2 = spool.tile([1, SQ], F32, name="ln_den2", tag="lnden", bufs=2)
                    nc.scalar.activation(
                        ln_den2[:], attn_pair[1][DEPTH : DEPTH + 1, :],
                        mybir.ActivationFunctionType.Ln,
                    )
                for half in range(2):
                    h = 2 * pr + half
                    aun = spool.tile(
                        [DEPTH + 1, SQ], F32, name=f"aun{h}", tag=f"aun{half}"
                    )
                    nc.vector.tensor_copy(aun[:], attn_pair[half][0 : DEPTH + 1, :])
                    auns.append(aun)
                recips = []
                if last:
                    for half, lnd in ((0, ln_den), (1, ln_den2)):
                        recip = spool.tile(
                            [1, SQ], F32, name=f"recipl{half}", tag=f"recip{half}"
                        )
                        nc.scalar.activation(recip[:], lnd[:], EXP, scale=-1.0)
                        recips.append(recip)
                else:
                    dens = spool.tile([33, SQ], F32, name=f"dens{pr}", tag="dens", bufs=1)
                    nc.gpsimd.memset(dens[:], 1.0)
                    nc.vector.tensor_copy(dens[0:1, :], auns[0][DEPTH : DEPTH + 1, :])
                    nc.vector.tensor_copy(dens[32:33, :], auns[1][DEPTH : DEPTH + 1, :])
                    drec = spool.tile([33, SQ], F32, name=f"drec{pr}", tag="drec", bufs=1)
                    nc.vector.reciprocal(drec[:], dens[:])
                    recipb = spool.tile([1, SQ], F32, name=f"recipb{pr}", tag="recip1")
                    nc.vector.tensor_copy(recipb[:], drec[32:33, :])
                    recips = [drec[0:1, :], recipb[:]]
                for half in range(2):
                    h = 2 * pr + half
                    dst = attnT_sb[pr][half * DEPTH : (half + 1) * DEPTH, :]
                    bcast = spool.tile(
                        [DEPTH, SQ], F32, name=f"bcast{h}", tag=f"bcast{half}"
                    )
                    nc.gpsimd.partition_broadcast(bcast[:], recips[half][:])
                    nc.vector.tensor_mul(dst, auns[half][0:DEPTH, :], bcast[:])

            # ---- output projection: out[q, od] = attnT^T @ Wo ----
            for qt in range(NQT):
                ps = pspool.tile([P, SQ], F32, name=f"ops{qt}", tag="at", bufs=2)
                for k in range(KT4):
                    nc.tensor.matmul(
                        ps[:, :512],
                        attnT_sb[k][:, qt * P : (qt + 1) * P],
                        wo_sb[k][:],
                        start=(k == 0),
                        stop=(k == KT4 - 1),
                    )
                osb = opool.tile([P, D], F32, name=f"osb{qt}", tag="osb")
                nc.vector.tensor_copy(osb[:], ps[:, :512])
                nc.sync.dma_start(out[qt * P : (qt + 1) * P, :], osb[:])

    nc.compile()
    return nc


_CACHE: dict = {}


def get_nc():
    if "nc" not in _CACHE:
        _CACHE["nc"] = build_nc()
    return _CACHE["nc"]


def make_in_maps(x, y, W_q, W_k, W_v, W_o):
    bf = ml_dtypes.bfloat16
    x = np.ascontiguousarray(x, dtype=np.float32)
    y = np.ascontiguousarray(y, dtype=np.float32)
    wq = (np.asarray(W_q, dtype=np.float32) * np.float32(DEPTH**-0.5)).astype(bf)
    wk = np.asarray(W_k, dtype=np.float32).astype(bf)
    wv = np.asarray(W_v, dtype=np.float32).astype(bf)
    wo = np.asarray(W_o, dtype=np.float32).astype(bf)
    yT_cache = [np.ascontiguousarray(y[b].T).astype(bf) for b in range(B)]
    in_maps = []
    for c in range(N_CORES):
        b, half = c // 2, c % 2
        in_maps.append(
            {
                "xT": np.ascontiguousarray(
                    x[b, half * SQ : (half + 1) * SQ, :].T
                ).astype(bf),
                "yT": yT_cache[b],
                "wq": wq,
                "wk": wk,
                "wv": wv,
                "wo": wo,
            }
        )
    return in_maps


def assemble_out(results):
    out = np.empty((B, S, D), np.float32)
    for c in range(N_CORES):
        b, half = c // 2, c % 2
        out[b, half * SQ : (half + 1) * SQ, :] = results[c]["out"]
    return out


def kernel(x, y, W_q, W_k, W_v, W_o):
    nc = get_nc()
    in_maps = make_in_maps(x, y, W_q, W_k, W_v, W_o)
    res = run_bass_kernel_spmd(nc, in_maps, core_ids=list(range(N_CORES)))
    return assemble_out(res.results)


# revision 10
# speedup vs baseline: 1.0760x; 1.0760x over previous
"""Multi-head attention (B=4, S=2048, D=512, H=8) on 8 Trainium2 cores.

Sharding: core c = (batch b = c//2, query-half = c%2). Each core computes
1024 query rows of one batch over all 2048 keys and all 8 heads, producing
a disjoint slice of the output -> no inter-core reduction needed.

Per-core layout is fully "transposed land" (contraction dim on partitions):
  xT [512,1024], yT [512,2048] prepared (transposed, bf16) on host.
  QT = Wq^T @ xT   (Wq pre-scaled by depth^-0.5 on host)
  KT = Wk^T @ yT
  V  = y @ Wv in natural [keys, dim] layout, stored strided into
       V_aug [128, 8*65] with a ones column per head (row 64 of the
       attention matmul output then accumulates softmax denominators).

Schedule (v2 — ScalarE exp is the bottleneck engine at ~142us busy, so
everything is organized to start it early and never starve it):
  - DMA priority: wk+yT first, then wq+xT, then wv, wo.
  - Only KT/QT for head pair 0 are computed up front; first exp issues
    ~15us in. V tiles and later pairs' KT/QT projections are emitted
    inside the attention loops where the PE has slack (ScalarE-bound
    steady state leaves ~40% PE idle per iteration).
  - per head pair (2p, 2p+1): head A on partitions 0:64, head B on
    64:128 of shared KT/QT tiles; their logits matmuls target disjoint
    PE row groups and run concurrently.
       logits[kt,qb] = (KT tile)^T @ QT  (bf16 operands, fp32 PSUM)
       exp over [128, 1024] (ScalarE, PSUM -> SBUF bf16)
       attnT += V_aug^T @ PT, fp32 PSUM, accumulated over 16 key tiles.
  - pair-end normalization: evacuate both heads' [65,1024] PSUM to SBUF
    (releases the psum banks for the next pair), then per head:
    reciprocal_approx_fast on the denominator row (single DVE op, ~51
    ULP — vs ~7us for the exact iterative reciprocal), gpsimd
    partition_broadcast, DVE multiply -> attnT bf16. All off the
    critical path except for the last pair.
  - out = attnT^T @ Wo per 128-query tile -> DMA (fp32).
Softmax skips max-subtraction (logits ~ N(0,1); exp cannot overflow fp32).
Matmul operands are bf16 (1 cycle/row on the PE vs 2 for fp32); all PSUM
accumulation fp32. End-to-end RMS relative error vs fp32 ~4e-3.
"""

import numpy as np
import ml_dtypes

import concourse.bass as bass
import concourse.tile as tile
from concourse import bacc, mybir
from concourse.bass_utils import run_bass_kernel_spmd

F32 = mybir.dt.float32
BF16 = mybir.dt.bfloat16
EXP = mybir.ActivationFunctionType.Exp

B, S, D = 4, 2048, 512
H = 8
DEPTH = D // H  # 64
SQ = S // 2  # queries per core (1024)
SK = S  # keys per core (2048)
N_CORES = 8

P = 128
KT4 = D // P  # 4 contraction tiles for projections
NKT = SK // P  # 16 key tiles
NQT = SQ // P  # 8 query tiles
VAUG_W = H * (DEPTH + 1)  # 520


def build_nc():
    nc = bacc.Bacc("TRN2", target_bir_lowering=False, debug=False)

    xT = nc.dram_tensor("xT", [D, SQ], BF16, kind="ExternalInput").ap()
    yT = nc.dram_tensor("yT", [D, SK], BF16, kind="ExternalInput").ap()
    wq = nc.dram_tensor("wq", [D, D], BF16, kind="ExternalInput").ap()
    wk = nc.dram_tensor("wk", [D, D], BF16, kind="ExternalInput").ap()
    wv = nc.dram_tensor("wv", [D, D], BF16, kind="ExternalInput").ap()
    wo = nc.dram_tensor("wo", [D, D], BF16, kind="ExternalInput").ap()
    out = nc.dram_tensor("out", [SQ, D], F32, kind="ExternalOutput").ap()

    with tile.TileContext(nc) as tc:
        with (
            tc.tile_pool(name="acts", bufs=1) as apool,
            tc.tile_pool(name="ps", bufs=1, space="PSUM") as pspool,
            tc.tile_pool(name="pt", bufs=6) as ptpool,
            tc.tile_pool(name="small", bufs=2) as spool,
            tc.tile_pool(name="outsb", bufs=2) as opool,
        ):
            # ---- load inputs, in the order the compute needs them ----
            def load4(name, src, width):
                tiles = []
                for k in range(KT4):
                    t = apool.tile([P, width], BF16, name=f"{name}{k}", tag=f"{name}{k}")
                    nc.sync.dma_start(t[:], src[k * P : (k + 1) * P, :])
                    tiles.append(t)
                return tiles

            wk_sb = load4("wk", wk, D)
            yT_sb = load4("yt", yT, SK)
            wv_sb = load4("wv", wv, D)
            wq_sb = load4("wq", wq, D)
            xT_sb = load4("xt", xT, SQ)
            wo_sb = load4("wo", wo, D)

            ones_sb = apool.tile([P, H], F32, name="ones_sb", tag="ones", bufs=1)
            nc.vector.memset(ones_sb[:], 1.0)
            ones_v = ones_sb.rearrange("p (h c) -> p h c", h=H, c=1)

            # ---- projection emitters (each borrows one 'lg' psum slot) ----
            V_sb = [None] * NKT

            def emit_v(kt):
                t = apool.tile([P, VAUG_W], BF16, name=f"vaug{kt}", tag=f"vaug{kt}")
                ps = pspool.tile([P, SQ], F32, name=f"vps{kt}", tag="lg", bufs=3)
                for k in range(KT4):
                    nc.tensor.matmul(
                        ps[:, :512],
                        yT_sb[k][:, kt * P : (kt + 1) * P],
                        wv_sb[k][:],
                        start=(k == 0),
                        stop=(k == KT4 - 1),
                    )
                tv = t.rearrange("p (h c) -> p h c", h=H, c=DEPTH + 1)
                nc.vector.tensor_copy(
                    tv[:, :, 0:DEPTH],
                    ps[:, :512].rearrange("p (h c) -> p h c", h=H, c=DEPTH),
                )
                nc.vector.tensor_copy(tv[:, :, DEPTH : DEPTH + 1], ones_v)
                V_sb[kt] = t

            QT_sb = [None] * KT4
            KT_sb = [None] * KT4

            def emit_kt_half(p, kb):
                if KT_sb[p] is None:
                    KT_sb[p] = apool.tile(
                        [P, SK], BF16, name=f"ktsb{p}", tag=f"ktsb{p}"
                    )
                t = KT_sb[p]
                ps = pspool.tile([P, SQ], F32, name=f"ktps{p}_{kb}", tag="lg", bufs=3)
                for qb in range(2):
                    for k in range(KT4):
                        nc.tensor.matmul(
                            ps[:, qb * 512 : (qb + 1) * 512],
                            wk_sb[k][:, p * P : (p + 1) * P],
                            yT_sb[k][
                                :,
                                kb * SQ + qb * 512 : kb * SQ + (qb + 1) * 512,
                            ],
                            start=(k == 0),
                            stop=(k == KT4 - 1),
                        )
                nc.vector.tensor_copy(t[:, kb * SQ : (kb + 1) * SQ], ps[:])

            def emit_qt(p):
                ps = pspool.tile([P, SQ], F32, name=f"qtps{p}", tag="lg", bufs=3)
                for qb in range(SQ // 512):
                    for k in range(KT4):
                        nc.tensor.matmul(
                            ps[:, qb * 512 : (qb + 1) * 512],
                            wq_sb[k][:, p * P : (p + 1) * P],
                            xT_sb[k][:, qb * 512 : (qb + 1) * 512],
                            start=(k == 0),
                            stop=(k == KT4 - 1),
                        )
                t = apool.tile([P, SQ], BF16, name=f"qtsb{p}", tag=f"qtsb{p}")
                nc.vector.tensor_copy(t[:], ps[:])
                QT_sb[p] = t

            # ---- prologue: only pair 0's KT/QT, plus the first V tiles ----
            emit_kt_half(0, 0)
            emit_v(0)
            emit_v(1)
            emit_qt(0)
            emit_kt_half(0, 1)
            emit_v(2)
            emit_v(3)
            emit_v(4)
            emit_v(5)

            attnT_sb = []
            for p in range(KT4):
                t = apool.tile([P, SQ], BF16, name=f"attnt{p}", tag=f"attnt{p}")
                attnT_sb.append(t)

            # ---- attention: head-pair outer, query-phase (512 q) middle ----
            # With the query dim split into two 512-wide phases, the two
            # attention accumulators are [65,512] = one PSUM bank each, which
            # frees enough PSUM for THREE logits slots. The third slot is what
            # lets the V / KT / QT projection borrows proceed without ever
            # blocking the logits->exp stream (strict-FIFO engine queues turn
            # any slot wait into a ScalarE bubble).
            for pr in range(KT4):
                for phase in range(2):
                    q0 = phase * 512
                    attn_ph = []
                    for half in range(2):
                        h = 2 * pr + half
                        t = pspool.tile(
                            [DEPTH + 1, 512], F32, name=f"attnps{h}_{phase}",
                            tag="at", bufs=2,
                        )
                        attn_ph.append(t)
                    for kt in range(NKT):
                        lg = pspool.tile(
                            [P, SQ], F32, name=f"lg{pr}_{phase}_{kt}", tag="lg",
                            bufs=3,
                        )
                        for half in range(2):
                            nc.tensor.matmul(
                                lg[:, half * 512 : (half + 1) * 512],
                                KT_sb[pr][
                                    half * DEPTH : (half + 1) * DEPTH,
                                    kt * P : (kt + 1) * P,
                                ],
                                QT_sb[pr][
                                    half * DEPTH : (half + 1) * DEPTH,
                                    q0 : q0 + 512,
                                ],
                                start=True,
                                stop=True,
                            )
                        pt = ptpool.tile(
                            [P, SQ], BF16, name=f"pt{pr}_{phase}_{kt}", tag="pt"
                        )
                        nc.scalar.activation(pt[:], lg[:], EXP)

                        # Projection work rides the third 'lg' slot and the
                        # PE's wait-for-exp window.
                        if pr == 0 and phase == 0 and 6 <= kt + 6 < NKT:
                            emit_v(kt + 6)
                        if phase == 1 and pr + 1 < KT4:
                            if kt == 3:
                                emit_kt_half(pr + 1, 0)
                            elif kt == 7:
                                emit_kt_half(pr + 1, 1)
                            elif kt == 11:
                                emit_qt(pr + 1)

                        for half in range(2):
                            h = 2 * pr + half
                            nc.tensor.matmul(
                                attn_ph[half][:, :],
                                V_sb[kt][
                                    :, h * (DEPTH + 1) : (h + 1) * (DEPTH + 1)
                                ],
                                pt[:, half * 512 : (half + 1) * 512],
                                start=(kt == 0),
                                stop=(kt == NKT - 1),
                            )
                    # ---- phase-end normalization ----
                    # Evacuate both heads' PSUM first (releases the 'at'
                    # banks for the next phase), then a single batched DVE
                    # reciprocal (both heads' denominator rows gathered to
                    # partitions 0/32 of one tile; the iterative reciprocal
                    # is free-dim-bound so one [33,512] op costs the same as
                    # [1,512]) runs off the critical path. The very last
                    # phase uses the ScalarE exp(-ln(x)) chain instead (same
                    # table set as the softmax exp, ~9e-6 rel err): ScalarE
                    # is idle once the final exp retires and its chain is
                    # much shorter, shrinking the kernel tail.
                    last = pr == KT4 - 1 and phase == 1
                    auns = []
                    if last:
                        ln_dens = []
                        for half in range(2):
                            lnd = spool.tile(
                                [1, 512], F32, name=f"ln_den{half}", tag="lnden",
                                bufs=2,
                            )
                            nc.scalar.activation(
                                lnd[:], attn_ph[half][DEPTH : DEPTH + 1, :],
                                mybir.ActivationFunctionType.Ln,
                            )
                            ln_dens.append(lnd)
                    for half in range(2):
                        h = 2 * pr + half
                        aun = spool.tile(
                            [DEPTH + 1, 512], F32, name=f"aun{h}_{phase}",
                            tag=f"aun{half}",
                        )
                        nc.vector.tensor_copy(aun[:], attn_ph[half][:, :])
                        auns.append(aun)
                    recips = []
                    if last:
                        for half in range(2):
                            recip = spool.tile(
                                [1, 512], F32, name=f"recipl{half}",
                                tag=f"recip{half}",
                            )
                            nc.scalar.activation(
                                recip[:], ln_dens[half][:], EXP, scale=-1.0
                            )
                            recips.append(recip[:])
                    else:
                        dens = spool.tile(
                            [33, 512], F32, name=f"dens{pr}_{phase}", tag="dens",
                            bufs=2,
                        )
                        nc.gpsimd.memset(dens[:], 1.0)
                        nc.vector.tensor_copy(
                            dens[0:1, :], auns[0][DEPTH : DEPTH + 1, :]
                        )
                        nc.vector.tensor_copy(
                            dens[32:33, :], auns[1][DEPTH : DEPTH + 1, :]
                        )
                        drec = spool.tile(
                            [33, 512], F32, name=f"drec{pr}_{phase}", tag="drec",
                            bufs=2,
                        )
                        nc.vector.reciprocal(drec[:], dens[:])
                        recipb = spool.tile(
                            [1, 512], F32, name=f"recipb{pr}_{phase}", tag="recip1"
                        )
                        nc.vector.tensor_copy(recipb[:], drec[32:33, :])
                        recips = [drec[0:1, :], recipb[:]]
                    for half in range(2):
                        h = 2 * pr + half
                        dst = attnT_sb[pr][
                            half * DEPTH : (half + 1) * DEPTH, q0 : q0 + 512
                        ]
                        bcast = spool.tile(
                            [DEPTH, 512], F32, name=f"bcast{h}_{phase}",
                            tag=f"bcast{half}",
                        )
                        nc.gpsimd.partition_broadcast(bcast[:], recips[half])
                        nc.vector.tensor_mul(dst, auns[half][0:DEPTH, :], bcast[:])

            # ---- output projection: out[q, od] = attnT^T @ Wo ----
            for qt in range(NQT):
                ps = pspool.tile([P, 512], F32, name=f"ops{qt}", tag="at", bufs=2)
                for k in range(KT4):
                    nc.tensor.matmul(
                        ps[:, :512],
                        attnT_sb[k][:, qt * P : (qt + 1) * P],
                        wo_sb[k][:],
                        start=(k == 0),
                        stop=(k == KT4 - 1),
                    )
                osb = opool.tile([P, D], F32, name=f"osb{qt}", tag="osb")
                nc.vector.tensor_copy(osb[:], ps[:, :512])
                nc.sync.dma_start(out[qt * P : (qt + 1) * P, :], osb[:])

    nc.compile()
    return nc


_CACHE: dict = {}


def get_nc():
    if "nc" not in _CACHE:
        _CACHE["nc"] = build_nc()
    return _CACHE["nc"]


def make_in_maps(x, y, W_q, W_k, W_v, W_o):
    bf = ml_dtypes.bfloat16
    x = np.ascontiguousarray(x, dtype=np.float32)
    y = np.ascontiguousarray(y, dtype=np.float32)
    wq = (np.asarray(W_q, dtype=np.float32) * np.float32(DEPTH**-0.5)).astype(bf)
    wk = np.asarray(W_k, dtype=np.float32).astype(bf)
    wv = np.asarray(W_v, dtype=np.float32).astype(bf)
    wo = np.asarray(W_o, dtype=np.float32).astype(bf)
    yT_cache = [np.ascontiguousarray(y[b].T).astype(bf) for b in range(B)]
    in_maps = []
    for c in range(N_CORES):
        b, half = c // 2, c % 2
        in_maps.append(
            {
                "xT": np.ascontiguousarray(
                    x[b, half * SQ : (half + 1) * SQ, :].T
                ).astype(bf),
                "yT": yT_cache[b],
                "wq": wq,
                "wk": wk,
                "wv": wv,
                "wo": wo,
            }
        )
    return in_maps


def assemble_out(results):
    out = np.empty((B, S, D), np.float32)
    for c in range(N_CORES):
        b, half = c // 2, c % 2
        out[b, half * SQ : (half + 1) * SQ, :] = results[c]["out"]
    return out


def kernel(x, y, W_q, W_k, W_v, W_o):
    nc = get_nc()
    in_maps = make_in_maps(x, y, W_q, W_k, W_v, W_o)
    res = run_bass_kernel_spmd(nc, in_maps, core_ids=list(range(N_CORES)))
    return assemble_out(res.results)


# revision 14
# speedup vs baseline: 1.0993x; 1.0216x over previous
"""Multi-head attention (B=4, S=2048, D=512, H=8) on 8 Trainium2 cores.

Sharding: core c = (batch b = c//2, query-half = c%2). Each core computes
1024 query rows of one batch over all 2048 keys and all 8 heads, producing
a disjoint slice of the output -> no inter-core reduction needed.

Per-core layout is fully "transposed land" (contraction dim on partitions):
  xT [512,1024], yT [512,2048] prepared (transposed, bf16) on host.
  QT = Wq^T @ xT   (Wq pre-scaled by depth^-0.5 on host)
  KT = Wk^T @ yT
  V  = y @ Wv in natural [keys, dim] layout, stored strided into
       V_aug [128, 8*65] with a ones column per head (row 64 of the
       attention matmul output then accumulates softmax denominators).

Schedule (v2 — ScalarE exp is the bottleneck engine at ~142us busy, so
everything is organized to start it early and never starve it):
  - DMA priority: wk+yT first, then wq+xT, then wv, wo.
  - Only KT/QT for head pair 0 are computed up front; first exp issues
    ~15us in. V tiles and later pairs' KT/QT projections are emitted
    inside the attention loops where the PE has slack (ScalarE-bound
    steady state leaves ~40% PE idle per iteration).
  - per head pair (2p, 2p+1): head A on partitions 0:64, head B on
    64:128 of shared KT/QT tiles; their logits matmuls target disjoint
    PE row groups and run concurrently.
       logits[kt,qb] = (KT tile)^T @ QT  (bf16 operands, fp32 PSUM)
       exp over [128, 1024] (ScalarE, PSUM -> SBUF bf16)
       attnT += V_aug^T @ PT, fp32 PSUM, accumulated over 16 key tiles.
  - pair-end normalization: evacuate both heads' [65,1024] PSUM to SBUF
    (releases the psum banks for the next pair), then per head:
    reciprocal_approx_fast on the denominator row (single DVE op, ~51
    ULP — vs ~7us for the exact iterative reciprocal), gpsimd
    partition_broadcast, DVE multiply -> attnT bf16. All off the
    critical path except for the last pair.
  - out = attnT^T @ Wo per 128-query tile -> DMA (fp32).
Softmax skips max-subtraction (logits ~ N(0,1); exp cannot overflow fp32).
Matmul operands are bf16 (1 cycle/row on the PE vs 2 for fp32); all PSUM
accumulation fp32. End-to-end RMS relative error vs fp32 ~4e-3.
"""

import numpy as np
import ml_dtypes

import concourse.bass as bass
import concourse.tile as tile
from concourse import bacc, mybir
from concourse.bass_utils import run_bass_kernel_spmd

F32 = mybir.dt.float32
BF16 = mybir.dt.bfloat16
EXP = mybir.ActivationFunctionType.Exp

B, S, D = 4, 2048, 512
H = 8
DEPTH = D // H  # 64
SQ = S // 2  # queries per core (1024)
SK = S  # keys per core (2048)
N_CORES = 8

P = 128
KT4 = D // P  # 4 contraction tiles for projections
NKT = SK // P  # 16 key tiles
NQT = SQ // P  # 8 query tiles
VAUG_W = H * (DEPTH + 1)  # 520


def build_nc():
    nc = bacc.Bacc("TRN2", target_bir_lowering=False, debug=False)

    xT = nc.dram_tensor("xT", [D, SQ], BF16, kind="ExternalInput").ap()
    yT = nc.dram_tensor("yT", [D, SK], BF16, kind="ExternalInput").ap()
    wq = nc.dram_tensor("wq", [D, D], BF16, kind="ExternalInput").ap()
    wk = nc.dram_tensor("wk", [D, D], BF16, kind="ExternalInput").ap()
    wv = nc.dram_tensor("wv", [D, D], BF16, kind="ExternalInput").ap()
    wo = nc.dram_tensor("wo", [D, D], BF16, kind="ExternalInput").ap()
    out = nc.dram_tensor("out", [SQ, D], F32, kind="ExternalOutput").ap()

    with tile.TileContext(nc) as tc:
        with (
            tc.tile_pool(name="acts", bufs=1) as apool,
            tc.tile_pool(name="ps", bufs=1, space="PSUM") as pspool,
            tc.tile_pool(name="pt", bufs=6) as ptpool,
            tc.tile_pool(name="small", bufs=2) as spool,
            tc.tile_pool(name="outsb", bufs=2) as opool,
        ):
            # ---- HAM warmup: the PE clock-gate opens only after ~3.4us of
            # sustained matmul activity, and the whole prologue otherwise
            # runs at the cold 1.2 GHz. Spam cheap N=64 matmuls into a
            # scratch PSUM slot while the first DMAs are in flight.
            warm = apool.tile([P, 64], BF16, name="warm", tag="warm", bufs=1)
            nc.vector.memset(warm[:], 0.0)
            wps = pspool.tile([P, SQ], F32, name="wps", tag="lg", bufs=3)
            for i in range(200):
                nc.tensor.matmul(wps[0:64, 0:64], warm[:], warm[:],
                                 start=True, stop=True)
            warm_anchor = apool.tile([1, 64], F32, name="warma", tag="warma", bufs=1)
            nc.vector.tensor_copy(warm_anchor[:], wps[0:1, 0:64])

            # ---- load inputs, in the order the compute needs them ----
            def load4(name, src, width, tiles=None, col0=0):
                made = tiles is None
                if made:
                    tiles = []
                for k in range(KT4):
                    if made:
                        t = apool.tile(
                            [P, width], BF16, name=f"{name}{k}", tag=f"{name}{k}"
                        )
                        tiles.append(t)
                    nc.sync.dma_start(
                        tiles[k][:, col0 : col0 + width],
                        src[k * P : (k + 1) * P, col0 : col0 + width],
                    )
                return tiles

            wk_sb = load4("wk", wk, D)
            yT_sb = [
                apool.tile([P, SK], BF16, name=f"yt{k}", tag=f"yt{k}")
                for k in range(KT4)
            ]
            load4("yt", yT, SQ, tiles=yT_sb, col0=0)  # first key half
            wq_sb = load4("wq", wq, D)
            xT_sb = load4("xt", xT, SQ)
            wv_sb = load4("wv", wv, D)
            load4("yt", yT, SQ, tiles=yT_sb, col0=SQ)  # second key half
            wo_sb = load4("wo", wo, D)

            ones_sb = apool.tile([P, H], F32, name="ones_sb", tag="ones", bufs=1)
            nc.vector.memset(ones_sb[:], 1.0)
            ones_v = ones_sb.rearrange("p (h c) -> p h c", h=H, c=1)

            # ---- projection emitters (each borrows one 'lg' psum slot) ----
            V_sb = [None] * NKT

            def emit_v(kt):
                t = apool.tile([P, VAUG_W], BF16, name=f"vaug{kt}", tag=f"vaug{kt}")
                ps = pspool.tile([P, SQ], F32, name=f"vps{kt}", tag="lg", bufs=3)
                for k in range(KT4):
                    nc.tensor.matmul(
                        ps[:, :512],
                        yT_sb[k][:, kt * P : (kt + 1) * P],
                        wv_sb[k][:],
                        start=(k == 0),
                        stop=(k == KT4 - 1),
                    )
                tv = t.rearrange("p (h c) -> p h c", h=H, c=DEPTH + 1)
                nc.vector.tensor_copy(
                    tv[:, :, 0:DEPTH],
                    ps[:, :512].rearrange("p (h c) -> p h c", h=H, c=DEPTH),
                )
                nc.vector.tensor_copy(tv[:, :, DEPTH : DEPTH + 1], ones_v)
                V_sb[kt] = t

            QT_sb = [None] * KT4
            KT_sb = [None] * KT4

            def emit_kt_half(p, kb):
                if KT_sb[p] is None:
                    KT_sb[p] = apool.tile(
                        [P, SK], BF16, name=f"ktsb{p}", tag=f"ktsb{p}"
                    )
                t = KT_sb[p]
                ps = pspool.tile([P, SQ], F32, name=f"ktps{p}_{kb}", tag="lg", bufs=3)
                for qb in range(2):
                    for k in range(KT4):
                        nc.tensor.matmul(
                            ps[:, qb * 512 : (qb + 1) * 512],
                            wk_sb[k][:, p * P : (p + 1) * P],
                            yT_sb[k][
                                :,
                                kb * SQ + qb * 512 : kb * SQ + (qb + 1) * 512,
                            ],
                            start=(k == 0),
                            stop=(k == KT4 - 1),
                        )
                nc.vector.tensor_copy(t[:, kb * SQ : (kb + 1) * SQ], ps[:])

            def emit_qt(p):
                ps = pspool.tile([P, SQ], F32, name=f"qtps{p}", tag="lg", bufs=3)
                for qb in range(SQ // 512):
                    for k in range(KT4):
                        nc.tensor.matmul(
                            ps[:, qb * 512 : (qb + 1) * 512],
                            wq_sb[k][:, p * P : (p + 1) * P],
                            xT_sb[k][:, qb * 512 : (qb + 1) * 512],
                            start=(k == 0),
                            stop=(k == KT4 - 1),
                        )
                t = apool.tile([P, SQ], BF16, name=f"qtsb{p}", tag=f"qtsb{p}")
                nc.vector.tensor_copy(t[:], ps[:])
                QT_sb[p] = t

            # ---- prologue: only pair 0's KT/QT, plus the first V tiles ----
            emit_kt_half(0, 0)
            emit_qt(0)
            emit_v(0)
            emit_v(1)
            emit_kt_half(0, 1)
            emit_v(2)
            emit_v(3)
            emit_v(4)
            emit_v(5)

            attnT_sb = []
            for p in range(KT4):
                t = apool.tile([P, SQ], BF16, name=f"attnt{p}", tag=f"attnt{p}")
                attnT_sb.append(t)

            # ---- output projection emitter (per 128-query tile) ----
            # qt 0..3 read only phase-0 columns of attnT and are emitted
            # inside pair 3 phase 1, overlapping the last attention phase.
            def emit_oproj(qt):
                ps = pspool.tile([P, 512], F32, name=f"ops{qt}", tag="lg", bufs=3)
                for k in range(KT4):
                    nc.tensor.matmul(
                        ps[:, :512],
                        attnT_sb[k][:, qt * P : (qt + 1) * P],
                        wo_sb[k][:],
                        start=(k == 0),
                        stop=(k == KT4 - 1),
                    )
                osb = opool.tile([P, D], F32, name=f"osb{qt}", tag="osb")
                nc.vector.tensor_copy(osb[:], ps[:, :512])
                nc.sync.dma_start(out[qt * P : (qt + 1) * P, :], osb[:])

            # Projection / output work hosted inside the attention loops,
            # keyed by (pair, phase, kt). Each borrows the third 'lg' slot.
            hooks = {}
            for kt in range(10):
                hooks[(0, 0, kt)] = (lambda k=kt: emit_v(k + 6))
            hooks[(0, 1, 1)] = lambda: emit_kt_half(1, 0)
            hooks[(0, 1, 5)] = lambda: emit_kt_half(1, 1)
            hooks[(0, 1, 9)] = lambda: emit_qt(1)
            for pr in (1, 2):
                hooks[(pr, 0, 3)] = (lambda p=pr: emit_kt_half(p + 1, 0))
                hooks[(pr, 0, 7)] = (lambda p=pr: emit_kt_half(p + 1, 1))
                hooks[(pr, 0, 11)] = (lambda p=pr: emit_qt(p + 1))
            for i, kt in enumerate((1, 4, 7, 10)):
                hooks[(3, 1, kt)] = (lambda q=i: emit_oproj(q))

            # ---- attention: head-pair outer, query-phase (512 q) middle ----
            # With the query dim split into two 512-wide phases, the two
            # attention accumulators are [65,512] = one PSUM bank each, which
            # frees enough PSUM for THREE logits slots. The third slot is what
            # lets the V / KT / QT projection borrows proceed without ever
            # blocking the logits->exp stream (strict-FIFO engine queues turn
            # any slot wait into a ScalarE bubble).
            for pr in range(KT4):
                for phase in range(2):
                    q0 = phase * 512
                    attn_ph = []
                    for half in range(2):
                        h = 2 * pr + half
                        t = pspool.tile(
                            [DEPTH + 1, 512], F32, name=f"attnps{h}_{phase}",
                            tag="at", bufs=2,
                        )
                        attn_ph.append(t)

                    # Logits+exp run 2 iterations ahead of PV so the exp
                    # stream never waits on a PSUM slot or the PE FIFO.
                    def emit_logits(kt, pr=pr, phase=phase, q0=q0):
                        lg = pspool.tile(
                            [P, SQ], F32, name=f"lg{pr}_{phase}_{kt}", tag="lg",
                            bufs=3,
                        )
                        for half in range(2):
                            nc.tensor.matmul(
                                lg[:, half * 512 : (half + 1) * 512],
                                KT_sb[pr][
                                    half * DEPTH : (half + 1) * DEPTH,
                                    kt * P : (kt + 1) * P,
                                ],
                                QT_sb[pr][
                                    half * DEPTH : (half + 1) * DEPTH,
                                    q0 : q0 + 512,
                                ],
                                start=True,
                                stop=True,
                            )
                        pt = ptpool.tile(
                            [P, SQ], BF16, name=f"pt{pr}_{phase}_{kt}", tag="pt"
                        )
                        nc.scalar.activation(pt[:], lg[:], EXP)
                        return pt

                    pts = {0: emit_logits(0), 1: emit_logits(1)}
                    for kt in range(NKT):
                        pt = pts.pop(kt)
                        for half in range(2):
                            h = 2 * pr + half
                            nc.tensor.matmul(
                                attn_ph[half][:, :],
                                V_sb[kt][
                                    :, h * (DEPTH + 1) : (h + 1) * (DEPTH + 1)
                                ],
                                pt[:, half * 512 : (half + 1) * 512],
                                start=(kt == 0),
                                stop=(kt == NKT - 1),
                            )
                        if kt + 2 < NKT:
                            pts[kt + 2] = emit_logits(kt + 2)
                        hook = hooks.get((pr, phase, kt))
                        if hook is not None:
                            hook()
                    # ---- phase-end normalization ----
                    # Evacuate both heads' PSUM first (releases the 'at'
                    # banks for the next phase), then a single batched DVE
                    # reciprocal (both heads' denominator rows gathered to
                    # partitions 0/32 of one tile; the iterative reciprocal
                    # is free-dim-bound so one [33,512] op costs the same as
                    # [1,512]) runs off the critical path. The very last
                    # phase uses the ScalarE exp(-ln(x)) chain instead (same
                    # table set as the softmax exp, ~9e-6 rel err): ScalarE
                    # is idle once the final exp retires and its chain is
                    # much shorter, shrinking the kernel tail.
                    last = pr == KT4 - 1 and phase == 1
                    auns = []
                    if last:
                        ln_dens = []
                        for half in range(2):
                            lnd = spool.tile(
                                [1, 512], F32, name=f"ln_den{half}", tag="lnden",
                                bufs=2,
                            )
                            nc.scalar.activation(
                                lnd[:], attn_ph[half][DEPTH : DEPTH + 1, :],
                                mybir.ActivationFunctionType.Ln,
                            )
                            ln_dens.append(lnd)
                    for half in range(2):
                        h = 2 * pr + half
                        aun = spool.tile(
                            [DEPTH + 1, 512], F32, name=f"aun{h}_{phase}",
                            tag=f"aun{half}",
                        )
                        nc.vector.tensor_copy(aun[:], attn_ph[half][:, :])
                        auns.append(aun)
                    recips = []
                    if last:
                        for half in range(2):
                            recip = spool.tile(
                                [1, 512], F32, name=f"recipl{half}",
                                tag=f"recip{half}",
                            )
                            nc.scalar.activation(
                                recip[:], ln_dens[half][:], EXP, scale=-1.0
                            )
                            recips.append(recip[:])
                    else:
                        dens = spool.tile(
                            [33, 512], F32, name=f"dens{pr}_{phase}", tag="dens",
                            bufs=2,
                        )
                        nc.gpsimd.memset(dens[:], 1.0)
                        nc.vector.tensor_copy(
                            dens[0:1, :], auns[0][DEPTH : DEPTH + 1, :]
                        )
                        nc.vector.tensor_copy(
                            dens[32:33, :], auns[1][DEPTH : DEPTH + 1, :]
                        )
                        drec = spool.tile(
                            [33, 512], F32, name=f"drec{pr}_{phase}", tag="drec",
                            bufs=2,
                        )
                        nc.vector.reciprocal(drec[:], dens[:])
                        recipb = spool.tile(
                            [1, 512], F32, name=f"recipb{pr}_{phase}", tag="recip1"
                        )
                        nc.vector.tensor_copy(recipb[:], drec[32:33, :])
                        recips = [drec[0:1, :], recipb[:]]
                    for half in range(2):
                        h = 2 * pr + half
                        dst = attnT_sb[pr][
                            half * DEPTH : (half + 1) * DEPTH, q0 : q0 + 512
                        ]
                        bcast = spool.tile(
                            [DEPTH, 512], F32, name=f"bcast{h}_{phase}",
                            tag=f"bcast{half}",
                        )
                        nc.gpsimd.partition_broadcast(bcast[:], recips[half])
                        nc.vector.tensor_mul(dst, auns[half][0:DEPTH, :], bcast[:])

            # ---- output projection tail: phase-1 query tiles ----
            for qt in range(NQT // 2, NQT):
                emit_oproj(qt)

    nc.compile()
    return nc


_CACHE: dict = {}


def get_nc():
    if "nc" not in _CACHE:
        _CACHE["nc"] = build_nc()
    return _CACHE["nc"]


def make_in_maps(x, y, W_q, W_k, W_v, W_o):
    bf = ml_dtypes.bfloat16
    x = np.ascontiguousarray(x, dtype=np.float32)
    y = np.ascontiguousarray(y, dtype=np.float32)
    wq = (np.asarray(W_q, dtype=np.float32) * np.float32(DEPTH**-0.5)).astype(bf)
    wk = np.asarray(W_k, dtype=np.float32).astype(bf)
    wv = np.asarray(W_v, dtype=np.float32).astype(bf)
    wo = np.asarray(W_o, dtype=np.float32).astype(bf)
    yT_cache = [np.ascontiguousarray(y[b].T).astype(bf) for b in range(B)]
    in_maps = []
    for c in range(N_CORES):
        b, half = c // 2, c % 2
        in_maps.append(
            {
                "xT": np.ascontiguousarray(
                    x[b, half * SQ : (half + 1) * SQ, :].T
                ).astype(bf),
                "yT": yT_cache[b],
                "wq": wq,
                "wk": wk,
                "wv": wv,
                "wo": wo,
            }
        )
    return in_maps


def assemble_out(results):
    out = np.empty((B, S, D), np.float32)
    for c in range(N_CORES):
        b, half = c // 2, c % 2
        out[b, half * SQ : (half + 1) * SQ, :] = results[c]["out"]
    return out


def kernel(x, y, W_q, W_k, W_v, W_o):
    nc = get_nc()
    in_maps = make_in_maps(x, y, W_q, W_k, W_v, W_o)
    res = run_bass_kernel_spmd(nc, in_maps, core_ids=list(range(N_CORES)))
    return assemble_out(res.results)
